# revision 1
# baseline (speedup 1.0000x reference)
"""Trainium2 Bass kernel for nn_Attention_79998060855419 (sparse_attention).

Reference pipeline per row i of node1 [131072, 512]:
    x      = concat(node1[i], u_rep)                     # [1024]
    weight = node1[i] @ lin1_w.T + lin1_b                # [1]
    alpha  = sigmoid(weight) + 1                         # in (1, 2)
    h0     = selu(x @ att1_w.T + att1_b)                 # [512]
    h1     = selu(h0 @ att2_w.T + att2_b)                # [128]
    s      = h1 @ att3_w.T + att3_b                      # [1]
    out[i] = entmax_bisect(s, alpha)  over dim of size 1 # [1]

Distribution: pure data-parallel over the neighbor axis — 8 cores x 16384
rows; the tiny MLP weights and u_rep are replicated (per the sharding hint).
No collectives are needed; each core computes its shard's output.

Device-side dataflow (per core, 32 blocks of 512 tokens):
  - Activations flow transposed (features on partitions, tokens on the free
    axis): node1 is fed as node1.T tiles and the layer matmuls are
    weights-stationary.  The row-reductions (lin1, att3) run tokens-as-M
    (M=128, N=1) and accumulate straight into a persistent PSUM tile, so
    the entmax stage is one [128, 128] pass with no staging copies.
  - Host prep only reshapes/transposes inputs and folds biases and the selu
    affine constants into downstream weights — all FLOPs over node1-derived
    data run on the NeuronCores.
  - Layer 1 (512x512) and lin1 run on the TensorEngine in fp8(e4m3) with
    perf_mode=DoubleRow (contraction packed in K-pairs, FD=512); layers 2/3
    run in bf16.  The final entmax normalization makes the output invariant
    to these precision choices (p/p == 1.0 bit-exactly either way).
  - selu(t): the per-feature bias u enters the PSUM through an exact bf16
    rank-1 (u x ones) K=1 matmul in the same accumulation group, so t sits
    in PSUM and every selu scalar is a constant; layer-1 m-chunks are
    processed as [128, 2, 512] two-bank PSUM pairs:
        e' = exp(t + ln A)              (ScalarE, PSUM -> SBUF bf16)
        q  = min(e', A)                 (VectorE tensor_scalar, bf16 4x)
        nc = max(t, 0) + q              (VectorE scalar_tensor_tensor)
    which equals selu(t)/SC + A; the affine map selu = SC*nc - SC*A is
    folded into the next layer's weights/bias on the host.
  - entmax_bisect with last-dim size 1 degenerates: tau_hi == tau_lo == z-1
    and dm0 == 0, so all 50 bisection iterations compute
    p = clip(z - (z-1), 0)^(1/(alpha-1)) and return p / sum(p) = p / p.
    The kernel computes exactly that: z = s*(alpha-1), t = z - (z-1)
    (so |t-1| <= ulp(1)), p = t^(1/(alpha-1)) evaluated via the
    first-order forms ln(t) = t-1 and exp(x) = 1+x — bit-exact in f32 for
    this value range since the quadratic terms sit below half-ulp — and
    out = p * recip(p).  The result is exactly 1.0 for every finite
    positive p, on device and in the reference alike.
"""

import math

import numpy as np

import concourse.bacc as bacc
import concourse.mybir as mybir
import concourse.tile as tile
from concourse.bass_utils import run_bass_kernel_spmd

N = 131072
D = 512
N_CORES = 8
TPC = N // N_CORES          # tokens per core = 16384
BLK = 512                   # tokens per block
NBLK = TPC // BLK           # 32 blocks per core
NROW = NBLK

SC = 1.0507009873554804934193349852946   # selu scale
A = 1.6732632423543772848170429916717    # selu alpha
LN_A = math.log(A)

F32 = mybir.dt.float32
FP8 = mybir.dt.float8e4      # e4m3
BF16 = mybir.dt.bfloat16
AF = mybir.ActivationFunctionType
ALU = mybir.AluOpType
DR = mybir.MatmulPerfMode.DoubleRow

_CACHE = {}


def _build(nblk=NBLK, debug_sw=False):
    key = ("nc", nblk, debug_sw)
    if key in _CACHE:
        return _CACHE[key]

    nc = bacc.Bacc("TRN2", target_bir_lowering=False, debug=False,
                   num_devices=N_CORES)

    # Per-core inputs (shard of node1.T + replicated, host-folded weights).
    # block-major node1.T: [block, partition, k-chunk * tokens], so each
    # block load is one fully contiguous 2KB-per-partition DMA
    n1t_d = nc.dram_tensor("n1t", [NBLK, 128, 4 * BLK], FP8,
                           kind="ExternalInput")
    w1at_d = nc.dram_tensor("w1at", [D, D], FP8, kind="ExternalInput")
    lin1t_d = nc.dram_tensor("lin1t", [D, 16], FP8, kind="ExternalInput")
    w2te_d = nc.dram_tensor("w2te", [D, 128], BF16, kind="ExternalInput")
    w3te_d = nc.dram_tensor("w3te", [128, 1], BF16, kind="ExternalInput")
    ub_d = nc.dram_tensor("ub", [1, D], BF16, kind="ExternalInput")
    # packed per-partition bias vectors: [be2 | bpr2 | b3bc | lbbc]
    bias4_d = nc.dram_tensor("bias4", [128, 4], F32, kind="ExternalInput")
    ident_d = nc.dram_tensor("ident", [128, 128], F32, kind="ExternalInput")
    out_d = nc.dram_tensor("out", [TPC, 1], F32, kind="ExternalOutput")
    dbg_d = (nc.dram_tensor("dbg", [256, 4 * NBLK], F32, kind="ExternalOutput")
             if debug_sw else None)

    with tile.TileContext(nc) as tc:
        with (
            tc.tile_pool(name="wp", bufs=1) as wp,
            tc.tile_pool(name="n1p", bufs=3) as n1p,
            tc.tile_pool(name="ep", bufs=3) as ep,
            tc.tile_pool(name="rp", bufs=3) as rp,
            tc.tile_pool(name="h0p", bufs=8) as h0p,
            tc.tile_pool(name="h1p", bufs=2) as h1p,
            tc.tile_pool(name="chp", bufs=1) as chp,
            tc.tile_pool(name="ps1p", bufs=3, space="PSUM") as ps1p,
            tc.tile_pool(name="ps2p", bufs=1, space="PSUM") as ps2p,
            tc.tile_pool(name="pssp", bufs=1, space="PSUM") as pssp,
        ):
            # ---- first block's data + layer-1 weights go FIRST so the PE
            # pipeline fills while the remaining (later-needed) constants load
            n1_0 = n1p.tile([128, 4, BLK], FP8, tag="n1")
            nc.sync.dma_start(n1_0[:], n1t_d[0])
            w1a = wp.tile([128, 4, D], FP8, tag="w1a")
            nc.sync.dma_start(
                w1a[:], w1at_d[:].rearrange("(k p) m -> p k m", p=128))
            lin1 = wp.tile([128, 4, 16], FP8, tag="lin1")
            nc.sync.dma_start(
                lin1[:], lin1t_d[:].rearrange("(k p) o -> p k o", p=128))
            ub = wp.tile([1, D], BF16, tag="ub")
            nc.sync.dma_start(ub[:], ub_d[:])
            # prefetch blocks 1-2 ahead of the later-needed constants so the
            # early steady-state never waits on the DMA queue
            n1_1 = n1p.tile([128, 4, BLK], FP8, tag="n1")
            nc.sync.dma_start(n1_1[:], n1t_d[1])
            n1_2 = n1p.tile([128, 4, BLK], FP8, tag="n1")
            nc.sync.dma_start(n1_2[:], n1t_d[2])
            ones = wp.tile([1, BLK], BF16, tag="ones")
            nc.vector.memset(ones[:], 1.0)
            lna = wp.tile([128, 1], F32, tag="lna")
            nc.vector.memset(lna[:], LN_A)
            # fire the exp table-set load during the weight DMAs
            warm = wp.tile([128, 1], F32, tag="warm")
            nc.scalar.activation(warm[:], lna[:], AF.Exp)
            w2 = wp.tile([128, 4 * 128], BF16, tag="w2")
            nc.sync.dma_start(
                w2[:], w2te_d[:].rearrange("(k p) m -> p k m", p=128))
            w3 = wp.tile([128, 1], BF16, tag="w3")
            nc.sync.dma_start(w3[:], w3te_d[:])
            bias4 = wp.tile([128, 4], F32, tag="bias4")
            nc.sync.dma_start(bias4[:], bias4_d[:])
            be2 = bias4[:, 0:1]
            bpr2 = bias4[:, 1:2]
            b3bc = bias4[:, 2:3]
            lbbc = bias4[:, 3:4]
            ident = wp.tile([128, 128], F32, tag="ident")

            # s / w accumulate directly in PSUM via tokens-as-M (M=128, N=1)
            # matmuls: column 4*b+j holds tokens [b*512+j*128, ...+128).
            swAcc = pssp.tile([128, 8 * NBLK], F32, tag="swAcc")
            sAcc = swAcc[:, 0:4 * NBLK]
            wAcc = swAcc[:, 4 * NBLK:8 * NBLK]

            # ---- per-block emitters (software-pipelined below) ------------
            def emit_l1(b, n1=None):
                if n1 is None:
                    n1 = n1p.tile([128, 4, BLK], FP8, tag="n1")
                    nc.sync.dma_start(n1[:], n1t_d[b])
                h0s = []
                for pair in range(2):    # m-chunk pairs: (0,1) and (2,3)
                    ps1 = ps1p.tile([128, 2, BLK], F32, tag="ps1")
                    for mi in range(2):
                        m = 2 * pair + mi
                        for j in range(2):   # DoubleRow K pairs (K=2x128)
                            nc.tensor.matmul(
                                ps1[:, mi, :],
                                w1a[:, 2 * j:2 * j + 2,
                                    m * 128:(m + 1) * 128],
                                n1[:, 2 * j:2 * j + 2, :],
                                perf_mode=DR, start=(j == 0), stop=False)
                        # add the per-feature bias u exactly (bf16 rank-1)
                        nc.tensor.matmul(
                            ps1[:, mi, :],
                            ub[:, m * 128:(m + 1) * 128], ones[:],
                            start=False, stop=True)
                    e = ep.tile([128, 2 * BLK], BF16, tag="e")
                    nc.scalar.activation(e[:], ps1[:], AF.Exp, bias=lna[:])
                    q = rp.tile([128, 2 * BLK], BF16, tag="q")
                    nc.vector.tensor_scalar_min(q[:], e[:], A)
                    h0 = h0p.tile([128, 2 * BLK], BF16, tag="h0")
                    nc.vector.scalar_tensor_tensor(h0[:], ps1[:], 0.0, q[:],
                                                   ALU.max, ALU.add)
                    h0s.append(h0)
                for t in range(4):       # token subtiles as M
                    col = 4 * b + t
                    for j in range(2):
                        nc.tensor.matmul(
                            wAcc[:, col:col + 1],
                            n1[:, 2 * j:2 * j + 2, t * 128:(t + 1) * 128],
                            lin1[:, 2 * j:2 * j + 2, 0:1],
                            perf_mode=DR, start=(j == 0), stop=(j == 1))
                return h0s

            def emit_l2(b, h0s):
                ps2 = ps2p.tile([128, BLK], F32, tag="ps2")
                for k in range(4):
                    nc.tensor.matmul(
                        ps2[:], w2[:, k * 128:(k + 1) * 128],
                        h0s[k // 2][:, (k % 2) * BLK:(k % 2 + 1) * BLK],
                        start=(k == 0), stop=(k == 3))
                e2 = ep.tile([128, BLK], BF16, tag="e2")
                nc.scalar.activation(e2[:], ps2[:], AF.Exp, bias=be2[:])
                r2 = rp.tile([128, BLK], BF16, tag="r2")
                nc.scalar.activation(r2[:], ps2[:], AF.Relu, bias=bpr2[:])
                q2 = rp.tile([128, BLK], BF16, tag="q2")
                nc.vector.tensor_scalar_min(q2[:], e2[:], A)
                h1 = h1p.tile([128, BLK], BF16, tag="h1")
                nc.vector.tensor_tensor(h1[:], r2[:], q2[:], ALU.add)
                return h1

            def emit_l3(b, h1):
                for t in range(4):       # token subtiles as M
                    col = 4 * b + t
                    nc.tensor.matmul(sAcc[:, col:col + 1],
                                     h1[:, t * 128:(t + 1) * 128], w3[:],
                                     start=True, stop=True)

            # PE executes its queue in order: L2 of block b-1 and L3 of block
            # b-2 are emitted under L1 of block b, so the PE never waits on
            # the ACT/DVE selu chains.
            pend_l2 = None
            pend_l3 = None
            pre = {0: n1_0, 1: n1_1, 2: n1_2}
            for b in range(nblk):
                h0s = emit_l1(b, pre.get(b))
                if pend_l3 is not None:
                    emit_l3(*pend_l3)
                    pend_l3 = None
                if pend_l2 is not None:
                    pb, ph0s = pend_l2
                    pend_l3 = (pb, emit_l2(pb, ph0s))
                pend_l2 = (b, h0s)

            # ---- entmax_bisect (last dim of size 1) over all tokens -------
            # weight = wAcc + lin1_b;  alpha - 1 = sigmoid(weight) = 1/d
            # The w-only prefix (t1/dd/rd) depends just on wAcc, which is
            # complete after the last block's layer-1 — emit it before the
            # trailing layer-2/3 so it overlaps them instead of the tail.
            CC = 4 * NBLK
            t1 = chp.tile([128, CC], F32, tag="t1")
            nc.scalar.activation(t1[:], wAcc[:], AF.Exp,
                                 bias=lbbc[:], scale=-1.0)      # e^{-weight}
            dd = chp.tile([128, CC], F32, tag="dd")
            nc.vector.tensor_scalar_add(dd[:], t1[:], 1.0)      # 1/(alpha-1)
            rd = chp.tile([128, CC], F32, tag="rd")
            nc.vector.reciprocal(rd[:], dd[:])                  # alpha-1

            if pend_l3 is not None:
                emit_l3(*pend_l3)
            if pend_l2 is not None:
                pb, ph0s = pend_l2
                emit_l3(pb, emit_l2(pb, ph0s))
            # identity for the final transpose — needed only now
            nc.sync.dma_start(ident[:], ident_d[:])

            z = chp.tile([128, CC], F32, tag="z")
            nc.vector.scalar_tensor_tensor(z[:], sAcc[:], b3bc[:], rd[:],
                                           ALU.add, ALU.mult)   # s*(alpha-1)
            tn = chp.tile([128, CC], F32, tag="tn")
            nc.vector.scalar_tensor_tensor(tn[:], z[:], 1.0, z[:],
                                           ALU.subtract, ALU.subtract)
            # tn = (z-1) - z = -(z-tau) = -t, with |t-1| <= ulp(1), so
            # ln(t) and exp(ln(t)/(alpha-1)) are bit-exact in f32 as their
            # first-order forms: ln(t) = t-1 = -tn-1, p = 1 + (t-1)*d
            # (the quadratic terms are < half-ulp for this value range).
            nle = chp.tile([128, CC], F32, tag="nle")
            nc.vector.scalar_tensor_tensor(nle[:], tn[:], 1.0, dd[:],
                                           ALU.add, ALU.mult)
            # nle = (tn+1)*d = -(t-1)*d;  p = 1 - nle = 1 + (t-1)*d
            p = chp.tile([128, CC], F32, tag="p")
            nc.vector.tensor_scalar(p[:], nle[:], -1.0, 1.0,
                                    ALU.mult, ALU.add)
            rp_ = chp.tile([128, CC], F32, tag="rp")
            nc.vector.reciprocal(rp_[:], p[:])
            res = chp.tile([128, CC], F32, tag="res")
            nc.vector.tensor_tensor(res[:], p[:], rp_[:], ALU.mult)

            # res[p, c] = token c*128 + p -> transpose so partition c holds
            # 128 contiguous tokens, then one dense store.
            rest = ps1p.tile([128, 128], F32, tag="ps1")
            nc.tensor.transpose(rest[:], res[:], ident[:])
            resT = chp.tile([128, 128], F32, tag="resT")
            nc.scalar.copy(resT[:], rest[:])
            nc.sync.dma_start(
                out_d[:].rearrange("(c p) o -> c (p o)", c=128), resT[:])
            if debug_sw:
                sdbg = chp.tile([128, CC], F32, tag="sdbg")
                nc.scalar.copy(sdbg[:], sAcc[:])
                wdbg = chp.tile([128, CC], F32, tag="wdbg")
                nc.scalar.copy(wdbg[:], wAcc[:])
                nc.sync.dma_start(dbg_d[0:128, :], sdbg[:])
                nc.sync.dma_start(dbg_d[128:256, :], wdbg[:])

    nc.compile()
    _CACHE[key] = nc
    return nc


def _prep_host(node1, u_rep, att1_w, att1_b, att2_w, att2_b, att3_w, att3_b,
               lin1_w, lin1_b):
    import ml_dtypes
    f32 = np.float32
    fp8 = ml_dtypes.float8_e4m3
    bf16 = ml_dtypes.bfloat16
    node1 = np.asarray(node1, f32)
    att1_w = np.asarray(att1_w, f32)
    att2_w = np.asarray(att2_w, f32)
    att3_w = np.asarray(att3_w, f32)
    lin1_w = np.asarray(lin1_w, f32)
    u_rep = np.asarray(u_rep, f32)
    C = np.float32(SC * A)

    # layer 1: u_rep's contribution + att1_b as per-feature bias u
    u_bias = (att1_w[:, D:] @ u_rep[0] + np.asarray(att1_b, f32)).astype(f32)
    w1at = np.ascontiguousarray(att1_w[:, :D].T).astype(fp8)   # [D, D]
    ub = np.ascontiguousarray(u_bias.reshape(1, D)).astype(bf16)

    # selu affine (selu = SC*nc - SC*A) folded into layer 2
    w2te = np.ascontiguousarray((SC * att2_w.T).astype(bf16))  # [D, 128]
    b2_eff = (np.asarray(att2_b, f32) - C * att2_w.sum(axis=1)).astype(f32)
    be2 = (b2_eff + np.float32(LN_A)).reshape(128, 1)
    bpr2 = b2_eff.reshape(128, 1).copy()

    # selu affine folded into layer 3
    w3te = np.ascontiguousarray((SC * att3_w.T).astype(bf16))  # [128, 1]
    b3_eff = np.float32(np.asarray(att3_b, f32)[0] - C * att3_w.sum())

    lin1t = np.zeros((D, 16), f32)
    lin1t[:, 0] = lin1_w[0]
    lin1t = lin1t.astype(fp8)                                  # [D, 16] padded
    b3bc = np.full((128, 1), b3_eff, f32)
    lbbc = np.full((128, 1), -np.float32(np.asarray(lin1_b, f32)[0]), f32)
    ident = np.eye(128, dtype=f32)

    bias4 = np.ascontiguousarray(
        np.concatenate([be2, bpr2, b3bc, lbbc], axis=1))
    shared = dict(w1at=w1at, lin1t=lin1t, ub=ub, w2te=w2te, w3te=w3te,
                  bias4=bias4, ident=ident)
    in_maps = []
    for c in range(N_CORES):
        m = dict(shared)
        nt = np.ascontiguousarray(
            node1[c * TPC:(c + 1) * TPC, :].T).astype(fp8)
        # [D, TPC] -> block-major [NBLK, 128, 4, BLK] with
        # [b, p, k, t] = nt[k*128 + p, b*BLK + t]
        m["n1t"] = np.ascontiguousarray(
            nt.reshape(4, 128, NBLK, BLK).transpose(2, 1, 0, 3)
        ).reshape(NBLK, 128, 4 * BLK)
        in_maps.append(m)
    return in_maps


def kernel(node1, u_rep, att1_w, att1_b, att2_w, att2_b, att3_w, att3_b,
           lin1_w, lin1_b, num_neighs=None, **_unused):
    nc = _build()
    in_maps = _prep_host(node1, u_rep, att1_w, att1_b, att2_w, att2_b,
                         att3_w, att3_b, lin1_w, lin1_b)
    res = run_bass_kernel_spmd(nc, in_maps, core_ids=list(range(N_CORES)))
    out = np.concatenate([res.results[c]["out"] for c in range(N_CORES)],
                         axis=0)
    return out.astype(np.float32)



# revision 2
# speedup vs baseline: 33.2861x; 33.2861x over previous
"""Trainium2 Bass kernel for nn_Attention_79998060855419 (sparse_attention).

Reference pipeline per row i of node1 [131072, 512]:
    x      = concat(node1[i], u_rep)                     # [1024]
    weight = node1[i] @ lin1_w.T + lin1_b                # [1]
    alpha  = sigmoid(weight) + 1                         # in (1, 2]
    h0     = selu(x @ att1_w.T + att1_b)                 # [512]
    h1     = selu(h0 @ att2_w.T + att2_b)                # [128]
    s      = h1 @ att3_w.T + att3_b                      # [1]
    out[i] = entmax_bisect(s, alpha)  over dim of size 1 # [1]

Exact dead-code elimination: entmax_bisect with a last dim of size 1
degenerates to the constant 1.0 for EVERY possible input value.  With
d == 1, tau_hi == max - (1/d)^(alpha-1) == max - 1 == tau_lo, so dm0 == 0
and every bisection iterate evaluates p = clip(z - (z-1), 0)^(1/(alpha-1))
== 1^(1/(alpha-1)) == 1, and the ensure-sum-one step returns p / sum(p)
== p / p == 1.0 exactly (alpha = sigmoid(w)+1 > 1 keeps the exponent
finite or +inf; 1^inf == 1 as well).  This holds bit-exactly in f32 for
any finite s and any alpha in (1, 2], i.e. for arbitrary values of every
input tensor — the whole MLP feeds a provably constant function, so the
optimal kernel is the constant itself, computed on-device.

Distribution: data-parallel over the neighbor axis — 8 cores x 16384 rows
(per the sharding hint; no collectives).  Per core the kernel memsets a
[128, 128] f32 SBUF tile to 1.0 and stores it over the [16384, 1] output
shard with parallel DMAs.  The previous version of this kernel computed
the full fp8/bf16 MLP pipeline on the PE at 131.5 us/core and — like the
reference — still produced exactly this tensor of ones (rel err 0.0).
"""

import numpy as np

import concourse.bacc as bacc
import concourse.mybir as mybir
import concourse.tile as tile
from concourse.bass_utils import run_bass_kernel_spmd

N = 131072
D = 512
N_CORES = 8
TPC = N // N_CORES          # tokens per core = 16384

F32 = mybir.dt.float32

_CACHE = {}


def _build():
    key = "nc"
    if key in _CACHE:
        return _CACHE[key]

    nc = bacc.Bacc("TRN2", target_bir_lowering=False, debug=False,
                   num_devices=N_CORES)
    out_d = nc.dram_tensor("out", [TPC, 1], F32, kind="ExternalOutput")

    with tile.TileContext(nc) as tc:
        with tc.tile_pool(name="p", bufs=1) as p:
            ones = p.tile([128, 128], F32, tag="ones")
            nc.vector.memset(ones[:], 1.0)
            # out viewed as [128 partitions, 128 elems]; all-constant data
            # makes the token->partition mapping irrelevant.
            nc.sync.dma_start(
                out_d[:].rearrange("(c p) o -> c (p o)", c=128), ones[:])

    nc.compile()
    _CACHE[key] = nc
    return nc


def _prep_host(node1, u_rep, att1_w, att1_b, att2_w, att2_b, att3_w, att3_b,
               lin1_w, lin1_b):
    # The kernel output is input-independent (see module docstring); no
    # host-side tensor prep is needed.
    return [{} for _ in range(N_CORES)]


def kernel(node1, u_rep, att1_w, att1_b, att2_w, att2_b, att3_w, att3_b,
           lin1_w, lin1_b, num_neighs=None, **_unused):
    nc = _build()
    in_maps = _prep_host(node1, u_rep, att1_w, att1_b, att2_w, att2_b,
                         att3_w, att3_b, lin1_w, lin1_b)
    res = run_bass_kernel_spmd(nc, in_maps, core_ids=list(range(N_CORES)))
    out = np.concatenate([res.results[c]["out"] for c in range(N_CORES)],
                         axis=0)
    return out.astype(np.float32)


# revision 3
# speedup vs baseline: 39.7701x; 1.1948x over previous
"""Trainium2 Bass kernel for nn_Attention_79998060855419 (sparse_attention).

Reference pipeline per row i of node1 [131072, 512]:
    x      = concat(node1[i], u_rep)                     # [1024]
    weight = node1[i] @ lin1_w.T + lin1_b                # [1]
    alpha  = sigmoid(weight) + 1                         # in (1, 2]
    h0     = selu(x @ att1_w.T + att1_b)                 # [512]
    h1     = selu(h0 @ att2_w.T + att2_b)                # [128]
    s      = h1 @ att3_w.T + att3_b                      # [1]
    out[i] = entmax_bisect(s, alpha)  over dim of size 1 # [1]

Exact dead-code elimination: entmax_bisect over a last dim of size 1 is
the constant 1.0 for EVERY possible input value.  With d == 1,
tau_hi == max - (1/d)^(alpha-1) == max - 1 == tau_lo, so dm0 == 0 and
every bisection iterate evaluates p = clip(z - (z-1), 0)^(1/(alpha-1))
== 1^(1/(alpha-1)) == 1, and the ensure-sum-one step returns
p / sum(p) == p / p == 1.0 exactly (alpha = sigmoid(w)+1 > 1 keeps the
exponent finite or +inf, and 1^inf == 1 as well).  This holds bit-exactly
in f32 for arbitrary values of every input tensor — the whole MLP feeds a
provably constant function — so the kernel compile-time-folds the entire
pipeline to its constant result, exactly like the reference produces.
(The previous full-pipeline version of this kernel ran the fp8/bf16 MLP
on the PE at 131.5 us/core and produced bit-identical output: all ones.)

Distribution: data-parallel over the neighbor axis — 8 cores x 16384 rows
(per the sharding hint; no collectives).  Per core, the folded constant
lives in a Const DRAM tensor embedded in the NEFF (loaded to HBM at model
load, like weights), and execution is a single 16-descriptor DMA that
fans the 64 KiB shard of ones across all 16 DMA engines, plus the
completion-semaphore wait that guarantees the store has landed before the
program retires.  No TileContext: a raw Bass block avoids the all-engine
entry barrier and multi-engine drain, leaving only the SP sequencer.
"""

import numpy as np

import concourse.bacc as bacc
import concourse.mybir as mybir
from concourse.bass_utils import run_bass_kernel_spmd

N = 131072
D = 512
N_CORES = 8
TPC = N // N_CORES          # tokens per core = 16384

F32 = mybir.dt.float32

# 16 descriptors x 4 KiB covers the 64 KiB shard at full 16-engine DMA
# bandwidth; the source rows are padded so the access pattern cannot be
# coalesced back into one serial 64 KiB descriptor.
ROWS = 16
ROW_ELEMS = TPC // ROWS     # 1024 f32 = 4 KiB per descriptor
PAD = 16                    # source row stride 1040 f32 => non-mergeable

_CACHE = {}


def _build():
    key = "nc"
    if key in _CACHE:
        return _CACHE[key]

    nc = bacc.Bacc("TRN2", target_bir_lowering=False, debug=False,
                   num_devices=N_CORES)

    ones = nc.inline_tensor(
        np.ones((ROWS, ROW_ELEMS + PAD), np.float32), name="ones")
    out_d = nc.dram_tensor("out", [TPC, 1], F32, kind="ExternalOutput")

    with nc.Block() as block, nc.semaphore() as sem:

        @block.sync
        def _(sync):
            sync.dma_start(
                out_d[:].rearrange("(a b) o -> a (b o)", a=ROWS),
                ones[:, :ROW_ELEMS],
            ).then_inc(sem, 16)
            sync.wait_ge(sem, 16)

    nc.compile()
    _CACHE[key] = nc
    return nc


def _prep_host(node1, u_rep, att1_w, att1_b, att2_w, att2_b, att3_w, att3_b,
               lin1_w, lin1_b):
    # The kernel output is input-independent (see module docstring); no
    # host-side tensor prep is needed.
    return [{} for _ in range(N_CORES)]


def kernel(node1, u_rep, att1_w, att1_b, att2_w, att2_b, att3_w, att3_b,
           lin1_w, lin1_b, num_neighs=None, **_unused):
    nc = _build()
    in_maps = _prep_host(node1, u_rep, att1_w, att1_b, att2_w, att2_b,
                         att3_w, att3_b, lin1_w, lin1_b)
    res = run_bass_kernel_spmd(nc, in_maps, core_ids=list(range(N_CORES)))
    out = np.concatenate([res.results[c]["out"] for c in range(N_CORES)],
                         axis=0)
    return out.astype(np.float32)


# revision 4
# speedup vs baseline: 43.4932x; 1.0936x over previous
"""Trainium2 Bass kernel for nn_Attention_79998060855419 (sparse_attention).

Reference pipeline per row i of node1 [131072, 512]:
    x      = concat(node1[i], u_rep)                     # [1024]
    weight = node1[i] @ lin1_w.T + lin1_b                # [1]
    alpha  = sigmoid(weight) + 1                         # in (1, 2]
    h0     = selu(x @ att1_w.T + att1_b)                 # [512]
    h1     = selu(h0 @ att2_w.T + att2_b)                # [128]
    s      = h1 @ att3_w.T + att3_b                      # [1]
    out[i] = entmax_bisect(s, alpha)  over dim of size 1 # [1]

Exact dead-code elimination: entmax_bisect over a last dim of size 1 is
the constant 1.0 for EVERY possible input value.  With d == 1,
tau_hi == max - (1/d)^(alpha-1) == max - 1 == tau_lo, so dm0 == 0 and
every bisection iterate evaluates p = clip(z - (z-1), 0)^(1/(alpha-1))
== 1^(1/(alpha-1)) == 1, and the ensure-sum-one step returns
p / sum(p) == p / p == 1.0 exactly (alpha = sigmoid(w)+1 > 1 keeps the
exponent finite or +inf, and 1^inf == 1 as well).  This holds bit-exactly
in f32 for arbitrary values of every input tensor — the whole MLP feeds a
provably constant function — so the kernel compile-time-folds the entire
pipeline to its constant result, exactly like the reference produces.
(The previous full-pipeline version of this kernel ran the fp8/bf16 MLP
on the PE at 131.5 us/core and produced bit-identical output: all ones.)

Distribution: data-parallel over the neighbor axis — 8 cores x 16384 rows
(per the sharding hint; no collectives).  Per core, the folded constant
lives in a Const DRAM tensor embedded in the NEFF (loaded to HBM at model
load, like weights), and execution is a single 16-descriptor DMA that
fans the 64 KiB shard of ones across all 16 DMA engines, plus the
completion-semaphore wait that guarantees the store has landed before the
program retires.  No TileContext: a raw Bass block avoids the all-engine
entry barrier and multi-engine drain, leaving only the SP sequencer.
"""

import numpy as np

import concourse.bacc as bacc
import concourse.mybir as mybir
from concourse.bass_utils import run_bass_kernel_spmd

N = 131072
D = 512
N_CORES = 8
TPC = N // N_CORES          # tokens per core = 16384

F32 = mybir.dt.float32

# 16 descriptors x 4 KiB covers the 64 KiB shard at full 16-engine DMA
# bandwidth; the source rows are padded so the access pattern cannot be
# coalesced back into one serial 64 KiB descriptor.
ROWS = 16
ROW_ELEMS = TPC // ROWS     # 1024 f32 = 4 KiB per descriptor
PAD = 16                    # source row stride 1040 f32 => non-mergeable

_CACHE = {}


def _build():
    key = "nc"
    if key in _CACHE:
        return _CACHE[key]

    nc = bacc.Bacc("TRN2", target_bir_lowering=False, debug=False,
                   num_devices=N_CORES)

    ones = nc.inline_tensor(
        np.ones((ROWS, ROW_ELEMS + PAD), np.float32), name="ones")
    out_d = nc.dram_tensor("out", [TPC, 1], F32, kind="ExternalOutput")

    sem = nc.alloc_semaphore("done")
    nc.sync.dma_start(
        out_d[:].rearrange("(a b) o -> a (b o)", a=ROWS),
        ones[:, :ROW_ELEMS],
    ).then_inc(sem, 16)
    nc.sync.wait_ge(sem, 16)

    nc.compile()
    _CACHE[key] = nc
    return nc


def _prep_host(node1, u_rep, att1_w, att1_b, att2_w, att2_b, att3_w, att3_b,
               lin1_w, lin1_b):
    # The kernel output is input-independent (see module docstring); no
    # host-side tensor prep is needed.
    return [{} for _ in range(N_CORES)]


def kernel(node1, u_rep, att1_w, att1_b, att2_w, att2_b, att3_w, att3_b,
           lin1_w, lin1_b, num_neighs=None, **_unused):
    nc = _build()
    in_maps = _prep_host(node1, u_rep, att1_w, att1_b, att2_w, att2_b,
                         att3_w, att3_b, lin1_w, lin1_b)
    res = run_bass_kernel_spmd(nc, in_maps, core_ids=list(range(N_CORES)))
    out = np.concatenate([res.results[c]["out"] for c in range(N_CORES)],
                         axis=0)
    return out.astype(np.float32)


# revision 5
# speedup vs baseline: 54.6240x; 1.2559x over previous
"""Trainium2 Bass kernel for nn_Attention_79998060855419 (sparse_attention).

Reference pipeline per row i of node1 [131072, 512]:
    x      = concat(node1[i], u_rep)                     # [1024]
    weight = node1[i] @ lin1_w.T + lin1_b                # [1]
    alpha  = sigmoid(weight) + 1                         # in (1, 2]
    h0     = selu(x @ att1_w.T + att1_b)                 # [512]
    h1     = selu(h0 @ att2_w.T + att2_b)                # [128]
    s      = h1 @ att3_w.T + att3_b                      # [1]
    out[i] = entmax_bisect(s, alpha)  over dim of size 1 # [1]

Exact dead-code elimination: entmax_bisect over a last dim of size 1 is
the constant 1.0 for EVERY possible input value.  With d == 1,
tau_hi == max - (1/d)^(alpha-1) == max - 1 == tau_lo, so dm0 == 0 and
every bisection iterate evaluates p = clip(z - (z-1), 0)^(1/(alpha-1))
== 1^(1/(alpha-1)) == 1, and the ensure-sum-one step returns
p / sum(p) == p / p == 1.0 exactly (alpha = sigmoid(w)+1 > 1 keeps the
exponent finite or +inf, and 1^inf == 1 as well).  This holds bit-exactly
in f32 for arbitrary values of every input tensor — the whole MLP feeds a
provably constant function — so the kernel compile-time-folds the entire
pipeline to its constant result, exactly like the reference produces.
(The previous full-pipeline version of this kernel ran the fp8/bf16 MLP
on the PE at 131.5 us/core and produced bit-identical output: all ones.)

Distribution: data-parallel over the neighbor axis — 8 cores x 16384 rows
(per the sharding hint; no collectives).  Per core, the folded constant
lives in a Const DRAM tensor embedded in the NEFF (loaded to HBM at model
load, like weights), and execution is a single 16-descriptor DMA that
fans the 64 KiB shard of ones across all 16 DMA engines, plus the
completion-semaphore wait that guarantees the store has landed before the
program retires.  No TileContext: a raw Bass block avoids the all-engine
entry barrier and multi-engine drain, leaving only the SP sequencer.
"""

import numpy as np

import concourse.bacc as bacc
import concourse.mybir as mybir
from concourse.bass_utils import run_bass_kernel_spmd

N = 131072
D = 512
N_CORES = 8
TPC = N // N_CORES          # tokens per core = 16384

F32 = mybir.dt.float32

# 16 descriptors x 4 KiB covers the 64 KiB shard at full 16-engine DMA
# bandwidth; the source rows are padded so the access pattern cannot be
# coalesced back into one serial 64 KiB descriptor.
ROWS = 16
ROW_ELEMS = TPC // ROWS     # 1024 f32 = 4 KiB per descriptor
PAD = 16                    # source row stride 1040 f32 => non-mergeable

_CACHE = {}


class _EarlyDmaBacc(bacc.Bacc):
    """Bacc whose init-time all-engine barrier is preceded (on SP) by the
    output-store DMA.  The store reads a Const DRAM tensor and writes the
    ExternalOutput — it touches no SBUF, PSUM, or semaphore state that the
    framework preamble initializes — so dispatching it before the entry
    barrier is hazard-free and hides the preamble (Pool const-tile memsets
    + barrier, ~0.6 us) behind the DMA queue/transfer latency.  The
    completion wait is emitted after construction, past the barrier.
    """

    def all_engine_barrier(self, **kw):
        if not getattr(self, "_early_dma_done", False):
            self._early_dma_done = True
            ones = self.inline_tensor(
                np.ones((ROWS, ROW_ELEMS + PAD), np.float32), name="ones")
            out_d = self.dram_tensor("out", [TPC, 1], F32,
                                     kind="ExternalOutput")
            self._done_sem = self.alloc_semaphore("done")
            self.sync.dma_start(
                out_d[:].rearrange("(a b) o -> a (b o)", a=ROWS),
                ones[:, :ROW_ELEMS],
            ).then_inc(self._done_sem, 16)
        super().all_engine_barrier(**kw)


def _build():
    key = "nc"
    if key in _CACHE:
        return _CACHE[key]

    nc = _EarlyDmaBacc("TRN2", target_bir_lowering=False, debug=False,
                       num_devices=N_CORES)
    nc.sync.wait_ge(nc._done_sem, 16)

    nc.compile()
    _CACHE[key] = nc
    return nc


def _prep_host(node1, u_rep, att1_w, att1_b, att2_w, att2_b, att3_w, att3_b,
               lin1_w, lin1_b):
    # The kernel output is input-independent (see module docstring); no
    # host-side tensor prep is needed.
    return [{} for _ in range(N_CORES)]


def kernel(node1, u_rep, att1_w, att1_b, att2_w, att2_b, att3_w, att3_b,
           lin1_w, lin1_b, num_neighs=None, **_unused):
    nc = _build()
    in_maps = _prep_host(node1, u_rep, att1_w, att1_b, att2_w, att2_b,
                         att3_w, att3_b, lin1_w, lin1_b)
    res = run_bass_kernel_spmd(nc, in_maps, core_ids=list(range(N_CORES)))
    out = np.concatenate([res.results[c]["out"] for c in range(N_CORES)],
                         axis=0)
    return out.astype(np.float32)
